# revision 12
# baseline (speedup 1.0000x reference)
"""CropAndResize (tf.image.crop_and_resize semantics) on 8 Trainium2 cores.

Strategy
--------
Data-parallel over the 32 boxes/images: each of the 8 cores processes 4
"slots".  Images are sorted by needed column-span and dealt round-robin so
slot s has a similar shape on every core (the compiled program is SPMD --
one program, per-core input *values*).

Per slot, on-device:
  1. One indirect DMA gathers, for each output row i, the input row pair
     (top_i, bot_i) restricted to the needed column window  ->  TB tile
     [128p, {g0_top, g0_bot, g1_top, g1_bot}, S*4].  (224 output rows are
     split in two partition groups g: i = p + 128*g.)
  2. Row lerp R = T*wt + B*wb.  The two scaled copies run on the Scalar
     engine (per-partition scale), the add on Vector.
  3. Column gather on GPSIMD (ap_gather): L block + R block, indices are
     per-image data.
  4. Column lerp O = L*wl + R*wr on Vector (wl/wr replicated [128, 224*4]
     weight tiles, validity folded in).
  5. DMA O out.

All indices/weights are computed on the host (cheap: 32*224 scalars) with
float32 ops exactly mirroring the reference so the validity masks and
floor() results match bit-for-bit.
"""

import math
import numpy as np

H = 1024
W = 1024
C = 4
CROP = 224
B = 32
NCORES = 8
SLOTS = B // NCORES  # 4
G = 2  # partition groups for 224 output rows: 128 + 96
ROW_ELEMS = W * C  # elements per input image row


# ----------------------------------------------------------------------------
# Host-side planning (exact float32 mirror of the reference index math)
# ----------------------------------------------------------------------------

def _axis_plan(lo, hi, n_in):
    """lo/hi: np.float32 box coords for one axis. Returns dict with
    t (low idx), b (high idx), wt, wb (validity-folded lerp weights)."""
    grid = np.arange(CROP, dtype=np.float32) / np.float32(CROP - 1)
    pos = (lo + grid * (hi - lo)) * np.float32(n_in - 1)
    valid = (pos >= 0) & (pos <= n_in - 1)
    low_f = np.floor(pos)
    lerp = pos - low_f
    t = np.clip(low_f.astype(np.int32), 0, n_in - 1)
    b = np.clip(t + 1, 0, n_in - 1)
    wt = np.where(valid, np.float32(1.0) - lerp, np.float32(0.0)).astype(np.float32)
    wb = np.where(valid, lerp, np.float32(0.0)).astype(np.float32)
    return t, b, wt, wb


def _plan_image(box):
    y1, x1, y2, x2 = (np.float32(box[0]), np.float32(box[1]),
                      np.float32(box[2]), np.float32(box[3]))
    ty, by, wty, wby = _axis_plan(y1, y2, H)
    tx, bx, wtx, wbx = _axis_plan(x1, x2, W)
    xlo = int(tx.min())
    xhi = int(bx.max())
    return dict(ty=ty, by=by, wty=wty, wby=wby,
                tx=tx, bx=bx, wtx=wtx, wbx=wbx,
                xlo=xlo, span=xhi - xlo + 1)


def _pad_span(s):
    return max(32, int(math.ceil(s / 8.0)) * 8)


def _wrap_idx16(idxlist):
    """ap_gather index layout: element m lives at [16k + m%16, m//16] for
    every 16-partition core block k (replicated)."""
    n = len(idxlist)
    assert n % 16 == 0
    wrapped = np.asarray(idxlist, dtype=np.int16).reshape(n // 16, 16).T  # [16, n//16]
    return np.tile(wrapped, (8, 1))  # [128, n//16]


def _build_host_inputs(x, boxes):
    """Returns (slot_shapes, in_maps, assignment) where assignment[c][s] is
    the image index handled by core c slot s."""
    plans = [_plan_image(boxes[b]) for b in range(B)]
    order = sorted(range(B), key=lambda b: -plans[b]["span"])
    assignment = [[-1] * SLOTS for _ in range(NCORES)]
    slot_shapes = []
    for s in range(SLOTS):
        grp = order[s * NCORES:(s + 1) * NCORES]
        for c in range(NCORES):
            assignment[c][s] = grp[c]
        slot_shapes.append(_pad_span(max(plans[b]["span"] for b in grp)))
    slot_shapes = tuple(slot_shapes)

    gix_w = max(s for s in ((448 // 16),)) if True else 28  # 448/16
    in_maps = []
    for c in range(NCORES):
        imgs = [assignment[c][s] for s in range(SLOTS)]
        ximg = np.ascontiguousarray(x[imgs]).reshape(-1)
        rix = np.zeros((SLOTS, 128, 4), dtype=np.int32)
        gix = np.zeros((SLOTS, 128, 448 // 16), dtype=np.int16)
        ylw = np.zeros((SLOTS, 128, 4), dtype=np.float32)
        xw = np.zeros((SLOTS, 2, 128, CROP * C), dtype=np.float32)
        for s in range(SLOTS):
            p = plans[imgs[s]]
            S = slot_shapes[s]
            xlo = min(p["xlo"], W - S)
            base = s * H * ROW_ELEMS + xlo * C
            # indirect row-pair gather indices: [p, {g0_t, g0_b, g1_t, g1_b}]
            for g in range(G):
                i = np.arange(128) + 128 * g
                i = np.minimum(i, CROP - 1)  # pad rows duplicate row 223
                pad = (np.arange(128) + 128 * g) >= CROP
                rix[s, :, 2 * g + 0] = base + p["ty"][i] * ROW_ELEMS
                rix[s, :, 2 * g + 1] = base + p["by"][i] * ROW_ELEMS
                ylw[s, :, 2 * g + 0] = np.where(pad, 0.0, p["wty"][i])
                ylw[s, :, 2 * g + 1] = np.where(pad, 0.0, p["wby"][i])
            # column gather indices (L block then R block), relative to xlo
            lrel = p["tx"] - xlo
            rrel = p["bx"] - xlo
            assert lrel.min() >= 0 and rrel.max() < S
            gix[s] = _wrap_idx16(np.concatenate([lrel, rrel]).astype(np.int16))
            # column weights replicated over partitions and channels
            xw[s, 0] = np.repeat(p["wtx"], C)[None, :]
            xw[s, 1] = np.repeat(p["wbx"], C)[None, :]
        in_maps.append({"ximg": ximg, "rix": rix, "gix": gix,
                        "ylw": ylw, "xw": xw})
    return slot_shapes, in_maps, assignment


# ----------------------------------------------------------------------------
# Device program
# ----------------------------------------------------------------------------

_PROGRAM_CACHE = {}


def _build_program(slot_shapes):
    if slot_shapes in _PROGRAM_CACHE:
        return _PROGRAM_CACHE[slot_shapes]

    import concourse.bass as bass
    import concourse.tile as tile
    from concourse import bacc, mybir

    f32 = mybir.dt.float32
    nc = bacc.Bacc("TRN2", target_bir_lowering=False, debug=False,
                   enable_asserts=False)

    tot = SLOTS * H * ROW_ELEMS
    ximg = nc.dram_tensor("ximg", [tot], f32, kind="ExternalInput").ap()
    rix = nc.dram_tensor("rix", [SLOTS, 128, 4], mybir.dt.int32,
                         kind="ExternalInput").ap()
    gix = nc.dram_tensor("gix", [SLOTS, 128, 448 // 16], mybir.dt.int16,
                         kind="ExternalInput").ap()
    ylw = nc.dram_tensor("ylw", [SLOTS, 128, 4], f32, kind="ExternalInput").ap()
    xw = nc.dram_tensor("xw", [SLOTS, 2, 128, CROP * C], f32,
                        kind="ExternalInput").ap()
    outp = nc.dram_tensor("out", [SLOTS, CROP, CROP * C], f32,
                          kind="ExternalOutput").ap()

    with tile.TileContext(nc) as tc:
        with (
            tc.tile_pool(name="meta", bufs=1) as meta_pool,
            tc.tile_pool(name="tb", bufs=1) as tb_pool,
            tc.tile_pool(name="mid", bufs=1) as mid_pool,
            tc.tile_pool(name="small", bufs=2) as small_pool,
        ):
            # Per-slot metadata, all loaded upfront (per-slot tags so slots
            # pipeline freely).
            metas = []
            for s in range(SLOTS):
                rix_sb = meta_pool.tile([128, 4], mybir.dt.int32, tag=f"rix{s}")
                nc.sync.dma_start(out=rix_sb[:], in_=rix[s])
                gix_sb = meta_pool.tile([128, 448 // 16], mybir.dt.int16,
                                        tag=f"gix{s}")
                nc.sync.dma_start(out=gix_sb[:], in_=gix[s])
                ylw_sb = meta_pool.tile([128, 4], f32, tag=f"ylw{s}")
                nc.sync.dma_start(out=ylw_sb[:], in_=ylw[s])
                wl_sb = meta_pool.tile([128, CROP * C], f32, tag=f"wl{s}")
                nc.sync.dma_start(out=wl_sb[:], in_=xw[s, 0])
                wr_sb = meta_pool.tile([128, CROP * C], f32, tag=f"wr{s}")
                nc.sync.dma_start(out=wr_sb[:], in_=xw[s, 1])
                metas.append((rix_sb, gix_sb, ylw_sb, wl_sb, wr_sb))

            # 1. gather rows from HBM, all slots up front so SWDGE
            # descriptor generation and the transfers stream ahead of the
            # compute.  HW indirect DMA supports one offset per partition,
            # so four gathers per slot: {g0,g1}x{top,bot}.
            # Issue SMALLEST slot first: Tile hands completion sems to 8
            # DMASW lanes round-robin, and DMA #9+ blocks the in-order Pool
            # stream until the lane's previous transfer fully completes --
            # small transfers recycle the lanes in microseconds.
            order = sorted(range(SLOTS), key=lambda s: slot_shapes[s])
            tbs = {}
            for s in order:
                F = slot_shapes[s] * C
                rix_sb = metas[s][0]
                TB = tb_pool.tile([128, 4, F], f32, tag=f"TB{s}")
                for j in range(4):
                    nc.gpsimd.indirect_dma_start(
                        out=TB[:, j],
                        out_offset=None,
                        in_=ximg.rearrange("(n o) -> n o", o=1),
                        in_offset=bass.IndirectOffsetOnAxis(
                            ap=rix_sb[:, j:j + 1], axis=0),
                    )
                tbs[s] = TB

            for s in order:
                S = slot_shapes[s]
                F = S * C
                rix_sb, gix_sb, ylw_sb, wl_sb, wr_sb = metas[s]
                TB = tbs[s]

                # 2. row lerp in place: TB[:,2g] = T*wt + B*wb.
                # Scaled copies on ScalarE (per-partition scale), adds on
                # VectorE.
                for g in range(G):
                    nc.scalar.mul(TB[:, 2 * g], TB[:, 2 * g],
                                  ylw_sb[:, 2 * g:2 * g + 1])
                    nc.scalar.mul(TB[:, 2 * g + 1], TB[:, 2 * g + 1],
                                  ylw_sb[:, 2 * g + 1:2 * g + 2])
                    nc.vector.tensor_add(TB[:, 2 * g], TB[:, 2 * g],
                                         TB[:, 2 * g + 1])

                # 3. column gather: [L(224) | R(224)] x C
                LR = mid_pool.tile([128, G, 2 * CROP * C], f32, tag=f"LR{s}")
                for g in range(G):
                    nc.gpsimd.ap_gather(
                        out_ap=LR[:, g].rearrange("p (n c) -> p n c", c=C),
                        in_ap=TB[:, 2 * g].rearrange("p (n c) -> p n c", c=C),
                        idxs_ap=gix_sb[:],
                        channels=128,
                        num_elems=S,
                        d=C,
                        num_idxs=2 * CROP,
                    )

                # 4. column lerp: O = L*wl + R*wr (R scaled in place)
                O = small_pool.tile([128, G, CROP * C], f32, tag="O")
                for g in range(G):
                    rblk = LR[:, g, CROP * C:2 * CROP * C]
                    nc.vector.tensor_tensor(
                        out=O[:, g], in0=LR[:, g, 0:CROP * C], in1=wl_sb[:],
                        op=mybir.AluOpType.mult)
                    nc.vector.tensor_tensor(
                        out=rblk, in0=rblk, in1=wr_sb[:],
                        op=mybir.AluOpType.mult)
                    nc.vector.tensor_add(O[:, g], O[:, g], rblk)

                # 5. store
                nc.sync.dma_start(out=outp[s, 0:128], in_=O[:, 0])
                nc.sync.dma_start(out=outp[s, 128:CROP], in_=O[0:CROP - 128, 1])

    nc.compile()
    _PROGRAM_CACHE[slot_shapes] = nc
    return nc


# ----------------------------------------------------------------------------
# Entry point
# ----------------------------------------------------------------------------

def _kernel_numpy_fallback(x, boxes, crop):
    b_idx = np.arange(B)
    grid = np.arange(crop, dtype=np.float32) / np.float32(crop - 1)
    y1, x1, y2, x2 = boxes[:, 0], boxes[:, 1], boxes[:, 2], boxes[:, 3]
    in_y = (y1[:, None] + grid[None, :] * (y2 - y1)[:, None]) * np.float32(H - 1)
    in_x = (x1[:, None] + grid[None, :] * (x2 - x1)[:, None]) * np.float32(W - 1)
    valid_y = (in_y >= 0) & (in_y <= H - 1)
    valid_x = (in_x >= 0) & (in_x <= W - 1)
    top_f = np.floor(in_y)
    left_f = np.floor(in_x)
    yl = (in_y - top_f)[:, :, None, None].astype(np.float32)
    xl = (in_x - left_f)[:, None, :, None].astype(np.float32)
    t = np.clip(top_f.astype(np.int32), 0, H - 1)
    b = np.clip(t + 1, 0, H - 1)
    l = np.clip(left_f.astype(np.int32), 0, W - 1)
    r = np.clip(l + 1, 0, W - 1)
    bi = b_idx[:, None, None]
    tl = x[bi, t[:, :, None], l[:, None, :]]
    tr = x[bi, t[:, :, None], r[:, None, :]]
    bl = x[bi, b[:, :, None], l[:, None, :]]
    br = x[bi, b[:, :, None], r[:, None, :]]
    top_i = tl + (tr - tl) * xl
    bot_i = bl + (br - bl) * xl
    out = top_i + (bot_i - top_i) * yl
    valid = (valid_y[:, :, None] & valid_x[:, None, :])[..., None]
    return np.where(valid, out, np.float32(0.0)).astype(np.float32)


def _run(x, boxes, trace=False, trace_cores=None):
    from concourse.bass_utils import run_bass_kernel_spmd

    slot_shapes, in_maps, assignment = _build_host_inputs(x, boxes)
    nc = _build_program(slot_shapes)
    res = run_bass_kernel_spmd(nc, in_maps, list(range(NCORES)),
                               trace=trace, trace_cores=trace_cores)

    out = np.empty((B, CROP, CROP, C), dtype=np.float32)
    for c in range(NCORES):
        core_out = res.results[c]["out"]  # [SLOTS, 224, 896]
        for s in range(SLOTS):
            out[assignment[c][s]] = core_out[s].reshape(CROP, CROP, C)
    return out, res


def kernel(x, boxes, out_im_res):
    x = np.asarray(x, dtype=np.float32)
    boxes = np.asarray(boxes, dtype=np.float32)
    crop = int(out_im_res)
    if x.shape != (B, H, W, C) or crop != CROP:
        return _kernel_numpy_fallback(x, boxes, crop)
    return _run(x, boxes)[0]


# revision 14
# speedup vs baseline: 1.4788x; 1.4788x over previous
"""CropAndResize (tf.image.crop_and_resize semantics) on 8 Trainium2 cores.

Strategy
--------
Data-parallel over the 32 boxes/images: each of the 8 cores processes 4
"slots".  Images are sorted by needed column-span and dealt round-robin so
slot s has a similar shape on every core (the compiled program is SPMD --
one program, per-core input *values*).

Per slot, on-device:
  1. Four indirect DMAs gather, for each output row i, the input rows
     top_i and bot_i restricted to the needed column window ->
     TB tile [128p, {g0_top, g0_bot, g1_top, g1_bot}, S*4].  (224 output
     rows split in two partition groups g: i = p + 128*g.)
  2. Row lerp in place: R_g = T_g*wt + B_g*wb.  Scaled copies on the
     Scalar engine (per-partition scale), adds on Vector.
  3. Column interp on the Tensor engine: for each channel c and each
     128-column chunk of the span, transpose R_g[:, x_chunk*4+c :: 4]
     (PE transpose -> PSUM -> copy to SBUF), then matmul against a
     host-built column-weight matrix Wx [x, j] (2 nonzeros per column j:
     the bilinear lerp weights, validity folded in), accumulating over
     x chunks in PSUM.
  4. Copy the [i, j] PSUM result into the channel-interleaved output tile
     and DMA out.

All indices/weights are computed on the host (cheap: 32*224 scalars) with
float32 ops exactly mirroring the reference so the validity masks and
floor() results match bit-for-bit.
"""

import math
import numpy as np

H = 1024
W = 1024
C = 4
CROP = 224
B = 32
NCORES = 8
SLOTS = B // NCORES  # 4
G = 2  # partition groups for 224 output rows: 128 + 96
ROW_ELEMS = W * C  # elements per input image row


# ----------------------------------------------------------------------------
# Host-side planning (exact float32 mirror of the reference index math)
# ----------------------------------------------------------------------------

def _axis_plan(lo, hi, n_in):
    grid = np.arange(CROP, dtype=np.float32) / np.float32(CROP - 1)
    pos = (lo + grid * (hi - lo)) * np.float32(n_in - 1)
    valid = (pos >= 0) & (pos <= n_in - 1)
    low_f = np.floor(pos)
    lerp = pos - low_f
    t = np.clip(low_f.astype(np.int32), 0, n_in - 1)
    b = np.clip(t + 1, 0, n_in - 1)
    wt = np.where(valid, np.float32(1.0) - lerp, np.float32(0.0)).astype(np.float32)
    wb = np.where(valid, lerp, np.float32(0.0)).astype(np.float32)
    return t, b, wt, wb


def _plan_image(box):
    y1, x1, y2, x2 = (np.float32(box[0]), np.float32(box[1]),
                      np.float32(box[2]), np.float32(box[3]))
    ty, by, wty, wby = _axis_plan(y1, y2, H)
    tx, bx, wtx, wbx = _axis_plan(x1, x2, W)
    xlo = int(tx.min())
    xhi = int(bx.max())
    return dict(ty=ty, by=by, wty=wty, wby=wby,
                tx=tx, bx=bx, wtx=wtx, wbx=wbx,
                xlo=xlo, span=xhi - xlo + 1)


def _pad_span(s):
    # span padded so the x dimension splits into 128-column chunks nicely;
    # minimum keeps indirect-DMA descriptors >= 512B.
    return max(128, int(math.ceil(s / 128.0)) * 128)


def _build_host_inputs(x, boxes):
    """Returns (slot_shapes, in_maps, assignment)."""
    plans = [_plan_image(boxes[b]) for b in range(B)]
    order = sorted(range(B), key=lambda b: -plans[b]["span"])
    assignment = [[-1] * SLOTS for _ in range(NCORES)]
    slot_shapes = []
    for s in range(SLOTS):
        grp = order[s * NCORES:(s + 1) * NCORES]
        for c in range(NCORES):
            assignment[c][s] = grp[c]
        slot_shapes.append(_pad_span(max(plans[b]["span"] for b in grp)))
    slot_shapes = tuple(slot_shapes)
    ks = [s // 128 for s in slot_shapes]
    ktot = sum(ks)

    in_maps = []
    for c in range(NCORES):
        imgs = [assignment[c][s] for s in range(SLOTS)]
        ximg = np.ascontiguousarray(x[imgs]).reshape(-1)
        rix = np.zeros((SLOTS, 128, 4), dtype=np.int32)
        ylw = np.zeros((SLOTS, 128, 4), dtype=np.float32)
        wxm = np.zeros((ktot * 128, CROP), dtype=np.float32)
        koff = 0
        for s in range(SLOTS):
            p = plans[imgs[s]]
            S = slot_shapes[s]
            xlo = min(p["xlo"], W - S)
            base = s * H * ROW_ELEMS + xlo * C
            for g in range(G):
                i = np.arange(128) + 128 * g
                i = np.minimum(i, CROP - 1)  # pad rows duplicate row 223
                pad = (np.arange(128) + 128 * g) >= CROP
                rix[s, :, 2 * g + 0] = base + p["ty"][i] * ROW_ELEMS
                rix[s, :, 2 * g + 1] = base + p["by"][i] * ROW_ELEMS
                ylw[s, :, 2 * g + 0] = np.where(pad, 0.0, p["wty"][i])
                ylw[s, :, 2 * g + 1] = np.where(pad, 0.0, p["wby"][i])
            # column weight matrix [S, 224]; l==r duplicates accumulate
            wx = np.zeros((S, CROP), dtype=np.float32)
            j = np.arange(CROP)
            np.add.at(wx, (p["tx"] - xlo, j), p["wtx"])
            np.add.at(wx, (p["bx"] - xlo, j), p["wbx"])
            wxm[koff * 128:(koff + ks[s]) * 128] = wx
            koff += ks[s]
        in_maps.append({"ximg": ximg, "rix": rix, "ylw": ylw, "wxm": wxm})
    return slot_shapes, in_maps, assignment


# ----------------------------------------------------------------------------
# Device program
# ----------------------------------------------------------------------------

_PROGRAM_CACHE = {}


def _build_program(slot_shapes):
    if slot_shapes in _PROGRAM_CACHE:
        return _PROGRAM_CACHE[slot_shapes]

    import concourse.bass as bass
    import concourse.tile as tile
    from concourse import bacc, mybir
    from concourse.masks import make_identity

    f32 = mybir.dt.float32
    nc = bacc.Bacc("TRN2", target_bir_lowering=False, debug=False,
                   enable_asserts=False)

    ks = [s // 128 for s in slot_shapes]
    ktot = sum(ks)
    tot = SLOTS * H * ROW_ELEMS
    ximg = nc.dram_tensor("ximg", [tot], f32, kind="ExternalInput").ap()
    rix = nc.dram_tensor("rix", [SLOTS, 128, 4], mybir.dt.int32,
                         kind="ExternalInput").ap()
    ylw = nc.dram_tensor("ylw", [SLOTS, 128, 4], f32, kind="ExternalInput").ap()
    wxm = nc.dram_tensor("wxm", [ktot * 128, CROP], f32,
                         kind="ExternalInput").ap()
    outp = nc.dram_tensor("out", [SLOTS, CROP, CROP * C], f32,
                          kind="ExternalOutput").ap()

    with tile.TileContext(nc) as tc:
        with (
            tc.tile_pool(name="meta", bufs=1) as meta_pool,
            tc.tile_pool(name="tb", bufs=1) as tb_pool,
            tc.tile_pool(name="rt", bufs=3) as rt_pool,
            tc.tile_pool(name="small", bufs=2) as small_pool,
            tc.tile_pool(name="pst", bufs=3, space="PSUM") as pst_pool,
            tc.tile_pool(name="pso", bufs=4, space="PSUM") as pso_pool,
        ):
            ident = meta_pool.tile([128, 128], f32, tag="ident")
            make_identity(nc, ident[:])

            # Per-slot metadata, all loaded upfront.
            metas = []
            koff = 0
            for s in range(SLOTS):
                rix_sb = meta_pool.tile([128, 4], mybir.dt.int32, tag=f"rix{s}")
                nc.sync.dma_start(out=rix_sb[:], in_=rix[s])
                ylw_sb = meta_pool.tile([128, 4], f32, tag=f"ylw{s}")
                nc.sync.dma_start(out=ylw_sb[:], in_=ylw[s])
                wx_sb = meta_pool.tile([128, ks[s], CROP], f32, tag=f"wx{s}")
                nc.sync.dma_start(
                    out=wx_sb[:],
                    in_=wxm[koff * 128:(koff + ks[s]) * 128].rearrange(
                        "(k p) j -> p k j", p=128))
                koff += ks[s]
                metas.append((rix_sb, ylw_sb, wx_sb))

            # 1. gather rows from HBM, all slots up front (smallest first:
            # Tile hands SWDGE completion sems to 8 lanes round-robin and
            # DMA #9+ blocks the in-order Pool stream until its lane's
            # previous transfer completes).
            order = sorted(range(SLOTS), key=lambda s: slot_shapes[s])
            tbs = {}
            for s in order:
                F = slot_shapes[s] * C
                rix_sb = metas[s][0]
                TB = tb_pool.tile([128, 4, F], f32, tag=f"TB{s}")
                for j in range(4):
                    nc.gpsimd.indirect_dma_start(
                        out=TB[:, j],
                        out_offset=None,
                        in_=ximg.rearrange("(n o) -> n o", o=1),
                        in_offset=bass.IndirectOffsetOnAxis(
                            ap=rix_sb[:, j:j + 1], axis=0),
                    )
                tbs[s] = TB

            for s in order:
                S = slot_shapes[s]
                K = ks[s]
                F = S * C
                rix_sb, ylw_sb, wx_sb = metas[s]
                TB = tbs[s]

                # 2. row lerp in place: TB[:,2g] = T_g*wt + B_g*wb
                for g in range(G):
                    nc.scalar.mul(TB[:, 2 * g], TB[:, 2 * g],
                                  ylw_sb[:, 2 * g:2 * g + 1])
                    nc.scalar.mul(TB[:, 2 * g + 1], TB[:, 2 * g + 1],
                                  ylw_sb[:, 2 * g + 1:2 * g + 2])
                    nc.vector.tensor_add(TB[:, 2 * g], TB[:, 2 * g],
                                         TB[:, 2 * g + 1])

                # 3. column interp per channel: transpose + matmul
                O = small_pool.tile([128, G, CROP * C], f32, tag=f"O{s}")
                for c in range(C):
                    pso = []
                    for g in range(G):
                        pso_t = pso_pool.tile([128, CROP], f32, tag="pso")
                        pso.append(pso_t)
                    for k in range(K):
                        # R_g[:, k*128*C + c :: C][:128] -> [128 x, i] chunks
                        rt = rt_pool.tile([128, CROP], f32, tag="rt")
                        for g in range(G):
                            pst = pst_pool.tile([128, 128], f32, tag="pst")
                            src = TB[:, 2 * g].rearrange(
                                "p (x c) -> p x c", c=C)[:, k * 128:(k + 1) * 128, c]
                            nc.tensor.transpose(
                                out=pst[:], in_=src, identity=ident[:])
                            ng = 128 if g == 0 else CROP - 128
                            nc.scalar.copy(rt[:, 128 * g:128 * g + ng],
                                           pst[:, :ng])
                        for g in range(G):
                            ng = 128 if g == 0 else CROP - 128
                            nc.tensor.matmul(
                                out=pso[g][:ng],
                                lhsT=rt[:, 128 * g:128 * g + ng],
                                rhs=wx_sb[:, k],
                                start=(k == 0),
                                stop=(k == K - 1),
                            )
                    # 4. interleave channel c into O
                    for g in range(G):
                        ng = 128 if g == 0 else CROP - 128
                        nc.vector.tensor_copy(
                            out=O[:ng, g].rearrange("p (j c) -> p j c", c=C)[:, :, c],
                            in_=pso[g][:ng])

                # 5. store
                nc.sync.dma_start(out=outp[s, 0:128], in_=O[:, 0])
                nc.sync.dma_start(out=outp[s, 128:CROP], in_=O[0:CROP - 128, 1])

    nc.compile()
    _PROGRAM_CACHE[slot_shapes] = nc
    return nc


# ----------------------------------------------------------------------------
# Entry point
# ----------------------------------------------------------------------------

def _kernel_numpy_fallback(x, boxes, crop):
    b_idx = np.arange(x.shape[0])
    grid = np.arange(crop, dtype=np.float32) / np.float32(crop - 1)
    y1, x1, y2, x2 = boxes[:, 0], boxes[:, 1], boxes[:, 2], boxes[:, 3]
    hh, ww = x.shape[1], x.shape[2]
    in_y = (y1[:, None] + grid[None, :] * (y2 - y1)[:, None]) * np.float32(hh - 1)
    in_x = (x1[:, None] + grid[None, :] * (x2 - x1)[:, None]) * np.float32(ww - 1)
    valid_y = (in_y >= 0) & (in_y <= hh - 1)
    valid_x = (in_x >= 0) & (in_x <= ww - 1)
    top_f = np.floor(in_y)
    left_f = np.floor(in_x)
    yl = (in_y - top_f)[:, :, None, None].astype(np.float32)
    xl = (in_x - left_f)[:, None, :, None].astype(np.float32)
    t = np.clip(top_f.astype(np.int32), 0, hh - 1)
    b = np.clip(t + 1, 0, hh - 1)
    l = np.clip(left_f.astype(np.int32), 0, ww - 1)
    r = np.clip(l + 1, 0, ww - 1)
    bi = b_idx[:, None, None]
    tl = x[bi, t[:, :, None], l[:, None, :]]
    tr = x[bi, t[:, :, None], r[:, None, :]]
    bl = x[bi, b[:, :, None], l[:, None, :]]
    br = x[bi, b[:, :, None], r[:, None, :]]
    top_i = tl + (tr - tl) * xl
    bot_i = bl + (br - bl) * xl
    out = top_i + (bot_i - top_i) * yl
    valid = (valid_y[:, :, None] & valid_x[:, None, :])[..., None]
    return np.where(valid, out, np.float32(0.0)).astype(np.float32)


def _run(x, boxes, trace=False, trace_cores=None):
    from concourse.bass_utils import run_bass_kernel_spmd

    slot_shapes, in_maps, assignment = _build_host_inputs(x, boxes)
    nc = _build_program(slot_shapes)
    res = run_bass_kernel_spmd(nc, in_maps, list(range(NCORES)),
                               trace=trace, trace_cores=trace_cores)

    out = np.empty((B, CROP, CROP, C), dtype=np.float32)
    for c in range(NCORES):
        core_out = res.results[c]["out"]  # [SLOTS, 224, 896]
        for s in range(SLOTS):
            out[assignment[c][s]] = core_out[s].reshape(CROP, CROP, C)
    return out, res


def kernel(x, boxes, out_im_res):
    x = np.asarray(x, dtype=np.float32)
    boxes = np.asarray(boxes, dtype=np.float32)
    crop = int(out_im_res)
    if x.shape != (B, H, W, C) or crop != CROP:
        return _kernel_numpy_fallback(x, boxes, crop)
    return _run(x, boxes)[0]


# revision 18
# speedup vs baseline: 1.5755x; 1.0654x over previous
"""CropAndResize (tf.image.crop_and_resize semantics) on 8 Trainium2 cores.

Strategy
--------
Data-parallel over the 32 boxes/images: each of the 8 cores processes 4
"slots".  Images are sorted by needed column-span and dealt round-robin so
slot s has a similar shape on every core (the compiled program is SPMD --
one program, per-core input *values*).

Per slot, on-device:
  1. Four indirect DMAs gather, for each output row i, the input rows
     top_i and bot_i restricted to the needed column window ->
     TB tile [128p, {g0_top, g0_bot, g1_top, g1_bot}, S*4].  (224 output
     rows split in two partition groups g: i = p + 128*g.)
  2. Row lerp in place: R_g = T_g*wt + B_g*wb.  Scaled copies on the
     Scalar engine (per-partition scale), adds on Vector.
  3. Column interp on the Tensor engine: for each channel c and each
     128-column chunk of the span, transpose R_g[:, x_chunk*4+c :: 4]
     (PE transpose -> PSUM -> copy to SBUF), then matmul against a
     host-built column-weight matrix Wx [x, j] (2 nonzeros per column j:
     the bilinear lerp weights, validity folded in), accumulating over
     x chunks in PSUM.
  4. Copy the [i, j] PSUM result into the channel-interleaved output tile
     and DMA out.

All indices/weights are computed on the host (cheap: 32*224 scalars) with
float32 ops exactly mirroring the reference so the validity masks and
floor() results match bit-for-bit.
"""

import math
import numpy as np

H = 1024
W = 1024
C = 4
CROP = 224
B = 32
NCORES = 8
SLOTS = B // NCORES  # 4
G = 2  # partition groups for 224 output rows: 128 + 96
ROW_ELEMS = W * C  # elements per input image row


# ----------------------------------------------------------------------------
# Host-side planning (exact float32 mirror of the reference index math)
# ----------------------------------------------------------------------------

def _axis_plan(lo, hi, n_in):
    grid = np.arange(CROP, dtype=np.float32) / np.float32(CROP - 1)
    pos = (lo + grid * (hi - lo)) * np.float32(n_in - 1)
    valid = (pos >= 0) & (pos <= n_in - 1)
    low_f = np.floor(pos)
    lerp = pos - low_f
    t = np.clip(low_f.astype(np.int32), 0, n_in - 1)
    b = np.clip(t + 1, 0, n_in - 1)
    wt = np.where(valid, np.float32(1.0) - lerp, np.float32(0.0)).astype(np.float32)
    wb = np.where(valid, lerp, np.float32(0.0)).astype(np.float32)
    return t, b, wt, wb


def _plan_image(box):
    y1, x1, y2, x2 = (np.float32(box[0]), np.float32(box[1]),
                      np.float32(box[2]), np.float32(box[3]))
    ty, by, wty, wby = _axis_plan(y1, y2, H)
    tx, bx, wtx, wbx = _axis_plan(x1, x2, W)
    xlo = int(tx.min())
    xhi = int(bx.max())
    return dict(ty=ty, by=by, wty=wty, wby=wby,
                tx=tx, bx=bx, wtx=wtx, wbx=wbx,
                xlo=xlo, span=xhi - xlo + 1)


def _pad_span(s):
    # span padded so the x dimension splits into 128-column chunks nicely;
    # minimum keeps indirect-DMA descriptors >= 512B.
    return max(128, int(math.ceil(s / 128.0)) * 128)


def _build_host_inputs(x, boxes):
    """Returns (slot_shapes, in_maps, assignment)."""
    plans = [_plan_image(boxes[b]) for b in range(B)]
    order = sorted(range(B), key=lambda b: -plans[b]["span"])
    assignment = [[-1] * SLOTS for _ in range(NCORES)]
    slot_shapes = []
    for s in range(SLOTS):
        grp = order[s * NCORES:(s + 1) * NCORES]
        for c in range(NCORES):
            assignment[c][s] = grp[c]
        slot_shapes.append(_pad_span(max(plans[b]["span"] for b in grp)))
    slot_shapes = tuple(slot_shapes)
    ks = [s // 128 for s in slot_shapes]
    ktot = sum(ks)

    in_maps = []
    for c in range(NCORES):
        imgs = [assignment[c][s] for s in range(SLOTS)]
        ximg = np.ascontiguousarray(x[imgs]).reshape(-1)
        rix = np.zeros((SLOTS, 128, 4), dtype=np.int32)
        ylw = np.zeros((SLOTS, 128, 4), dtype=np.float32)
        wxm = np.zeros((ktot * 128, CROP), dtype=np.float32)
        koff = 0
        for s in range(SLOTS):
            p = plans[imgs[s]]
            S = slot_shapes[s]
            xlo = min(p["xlo"], W - S)
            base = s * H * ROW_ELEMS + xlo * C
            for g in range(G):
                i = np.arange(128) + 128 * g
                i = np.minimum(i, CROP - 1)  # pad rows duplicate row 223
                pad = (np.arange(128) + 128 * g) >= CROP
                rix[s, :, 2 * g + 0] = base + p["ty"][i] * ROW_ELEMS
                rix[s, :, 2 * g + 1] = base + p["by"][i] * ROW_ELEMS
                ylw[s, :, 2 * g + 0] = np.where(pad, 0.0, p["wty"][i])
                ylw[s, :, 2 * g + 1] = np.where(pad, 0.0, p["wby"][i])
            # column weight matrix [S, 224]; l==r duplicates accumulate
            wx = np.zeros((S, CROP), dtype=np.float32)
            j = np.arange(CROP)
            np.add.at(wx, (p["tx"] - xlo, j), p["wtx"])
            np.add.at(wx, (p["bx"] - xlo, j), p["wbx"])
            wxm[koff * 128:(koff + ks[s]) * 128] = wx
            koff += ks[s]
        in_maps.append({"ximg": ximg, "rix": rix, "ylw": ylw, "wxm": wxm})
    return slot_shapes, in_maps, assignment


# ----------------------------------------------------------------------------
# Device program
# ----------------------------------------------------------------------------

_PROGRAM_CACHE = {}


def _build_program(slot_shapes):
    if slot_shapes in _PROGRAM_CACHE:
        return _PROGRAM_CACHE[slot_shapes]

    import concourse.bass as bass
    import concourse.tile as tile
    from concourse import bacc, mybir
    from concourse.masks import make_identity

    f32 = mybir.dt.float32
    nc = bacc.Bacc("TRN2", target_bir_lowering=False, debug=False,
                   enable_asserts=False)

    ks = [s // 128 for s in slot_shapes]
    ktot = sum(ks)
    tot = SLOTS * H * ROW_ELEMS
    ximg = nc.dram_tensor("ximg", [tot], f32, kind="ExternalInput").ap()
    rix = nc.dram_tensor("rix", [SLOTS, 128, 4], mybir.dt.int32,
                         kind="ExternalInput").ap()
    ylw = nc.dram_tensor("ylw", [SLOTS, 128, 4], f32, kind="ExternalInput").ap()
    wxm = nc.dram_tensor("wxm", [ktot * 128, CROP], f32,
                         kind="ExternalInput").ap()
    outp = nc.dram_tensor("out", [SLOTS, CROP, CROP * C], f32,
                          kind="ExternalOutput").ap()

    with tile.TileContext(nc) as tc:
        with (
            tc.tile_pool(name="meta", bufs=1) as meta_pool,
            tc.tile_pool(name="tb", bufs=1) as tb_pool,
            tc.tile_pool(name="rt", bufs=3) as rt_pool,
            tc.tile_pool(name="small", bufs=2) as small_pool,
            tc.tile_pool(name="pst", bufs=3, space="PSUM") as pst_pool,
            tc.tile_pool(name="pso", bufs=4, space="PSUM") as pso_pool,
        ):
            ident = meta_pool.tile([128, 128], f32, tag="ident")
            make_identity(nc, ident[:])

            # Per-slot metadata, all loaded upfront.
            metas = []
            koff = 0
            for s in range(SLOTS):
                rix_sb = meta_pool.tile([128, 4], mybir.dt.int32, tag=f"rix{s}")
                nc.sync.dma_start(out=rix_sb[:], in_=rix[s])
                ylw_sb = meta_pool.tile([128, 4], f32, tag=f"ylw{s}")
                nc.sync.dma_start(out=ylw_sb[:], in_=ylw[s])
                wx_sb = meta_pool.tile([128, ks[s], CROP], f32, tag=f"wx{s}")
                nc.sync.dma_start(
                    out=wx_sb[:],
                    in_=wxm[koff * 128:(koff + ks[s]) * 128].rearrange(
                        "(k p) j -> p k j", p=128))
                koff += ks[s]
                metas.append((rix_sb, ylw_sb, wx_sb))

            # 1. gather rows from HBM, all slots up front (smallest first:
            # Tile hands SWDGE completion sems to 8 lanes round-robin and
            # DMA #9+ blocks the in-order Pool stream until its lane's
            # previous transfer completes).
            order = sorted(range(SLOTS), key=lambda s: slot_shapes[s])
            tbs = {}
            for s in order:
                F = slot_shapes[s] * C
                rix_sb = metas[s][0]
                TB = tb_pool.tile([128, 4, F], f32, tag=f"TB{s}")
                for j in range(4):
                    nc.gpsimd.indirect_dma_start(
                        out=TB[:, j],
                        out_offset=None,
                        in_=ximg.rearrange("(n o) -> n o", o=1),
                        in_offset=bass.IndirectOffsetOnAxis(
                            ap=rix_sb[:, j:j + 1], axis=0),
                    )
                tbs[s] = TB

            for s in order:
                S = slot_shapes[s]
                K = ks[s]
                F = S * C
                rix_sb, ylw_sb, wx_sb = metas[s]
                TB = tbs[s]

                # 2. row lerp in place: TB[:,2g] = T_g*wt + B_g*wb
                # (T-mul on ScalarE, B-mul on VectorE tensor_scalar 2x mode,
                # add on VectorE)
                for g in range(G):
                    nc.scalar.mul(TB[:, 2 * g], TB[:, 2 * g],
                                  ylw_sb[:, 2 * g:2 * g + 1])
                    nc.vector.tensor_scalar_mul(
                        TB[:, 2 * g + 1], TB[:, 2 * g + 1],
                        ylw_sb[:, 2 * g + 1:2 * g + 2])
                    nc.vector.tensor_add(TB[:, 2 * g], TB[:, 2 * g],
                                         TB[:, 2 * g + 1])

                # 3. column interp per channel: all K transposes (PE->PSUM,
                # one merged copy per chunk to SBUF), then all matmuls, so
                # the PE stream never stalls on the PSUM-copy round trip.
                O = small_pool.tile([128, G, CROP * C], f32, tag=f"O{s}")
                for c in range(C):
                    rts = []
                    for k in range(K):
                        pst = pst_pool.tile([128, 256], f32, tag="pst")
                        for g in range(G):
                            src = TB[:, 2 * g].rearrange(
                                "p (x c) -> p x c", c=C)[:, k * 128:(k + 1) * 128, c]
                            nc.tensor.transpose(
                                out=pst[:, 128 * g:128 * (g + 1)],
                                in_=src,
                                identity=ident[:])
                        rt = rt_pool.tile([128, CROP], f32, tag="rt")
                        if k % 2 == 0:
                            nc.scalar.copy(rt[:], pst[:, :CROP])
                        else:
                            nc.vector.tensor_copy(out=rt[:], in_=pst[:, :CROP])
                        rts.append(rt)
                    pso = []
                    for g in range(G):
                        pso_t = pso_pool.tile([128, CROP], f32, tag="pso")
                        pso.append(pso_t)
                    for k in range(K):
                        for g in range(G):
                            ng = 128 if g == 0 else CROP - 128
                            nc.tensor.matmul(
                                out=pso[g][:ng],
                                lhsT=rts[k][:, 128 * g:128 * g + ng],
                                rhs=wx_sb[:, k],
                                start=(k == 0),
                                stop=(k == K - 1),
                            )
                    # 4. interleave channel c into O
                    for g in range(G):
                        ng = 128 if g == 0 else CROP - 128
                        ov = O[:ng, g].rearrange("p (j c) -> p j c", c=C)[:, :, c]
                        if g == 0:
                            nc.vector.tensor_copy(out=ov, in_=pso[g][:ng])
                        else:
                            nc.scalar.copy(ov, pso[g][:ng])

                # 5. store
                nc.sync.dma_start(out=outp[s, 0:128], in_=O[:, 0])
                nc.sync.dma_start(out=outp[s, 128:CROP], in_=O[0:CROP - 128, 1])

    nc.compile()
    _PROGRAM_CACHE[slot_shapes] = nc
    return nc


# ----------------------------------------------------------------------------
# Entry point
# ----------------------------------------------------------------------------

def _kernel_numpy_fallback(x, boxes, crop):
    b_idx = np.arange(x.shape[0])
    grid = np.arange(crop, dtype=np.float32) / np.float32(crop - 1)
    y1, x1, y2, x2 = boxes[:, 0], boxes[:, 1], boxes[:, 2], boxes[:, 3]
    hh, ww = x.shape[1], x.shape[2]
    in_y = (y1[:, None] + grid[None, :] * (y2 - y1)[:, None]) * np.float32(hh - 1)
    in_x = (x1[:, None] + grid[None, :] * (x2 - x1)[:, None]) * np.float32(ww - 1)
    valid_y = (in_y >= 0) & (in_y <= hh - 1)
    valid_x = (in_x >= 0) & (in_x <= ww - 1)
    top_f = np.floor(in_y)
    left_f = np.floor(in_x)
    yl = (in_y - top_f)[:, :, None, None].astype(np.float32)
    xl = (in_x - left_f)[:, None, :, None].astype(np.float32)
    t = np.clip(top_f.astype(np.int32), 0, hh - 1)
    b = np.clip(t + 1, 0, hh - 1)
    l = np.clip(left_f.astype(np.int32), 0, ww - 1)
    r = np.clip(l + 1, 0, ww - 1)
    bi = b_idx[:, None, None]
    tl = x[bi, t[:, :, None], l[:, None, :]]
    tr = x[bi, t[:, :, None], r[:, None, :]]
    bl = x[bi, b[:, :, None], l[:, None, :]]
    br = x[bi, b[:, :, None], r[:, None, :]]
    top_i = tl + (tr - tl) * xl
    bot_i = bl + (br - bl) * xl
    out = top_i + (bot_i - top_i) * yl
    valid = (valid_y[:, :, None] & valid_x[:, None, :])[..., None]
    return np.where(valid, out, np.float32(0.0)).astype(np.float32)


def _run(x, boxes, trace=False, trace_cores=None):
    from concourse.bass_utils import run_bass_kernel_spmd

    slot_shapes, in_maps, assignment = _build_host_inputs(x, boxes)
    nc = _build_program(slot_shapes)
    res = run_bass_kernel_spmd(nc, in_maps, list(range(NCORES)),
                               trace=trace, trace_cores=trace_cores)

    out = np.empty((B, CROP, CROP, C), dtype=np.float32)
    for c in range(NCORES):
        core_out = res.results[c]["out"]  # [SLOTS, 224, 896]
        for s in range(SLOTS):
            out[assignment[c][s]] = core_out[s].reshape(CROP, CROP, C)
    return out, res


def kernel(x, boxes, out_im_res):
    x = np.asarray(x, dtype=np.float32)
    boxes = np.asarray(boxes, dtype=np.float32)
    crop = int(out_im_res)
    if x.shape != (B, H, W, C) or crop != CROP:
        return _kernel_numpy_fallback(x, boxes, crop)
    return _run(x, boxes)[0]
